# revision 13
# baseline (speedup 1.0000x reference)
"""HGNN layer on 8 Trainium2 NeuronCores (Bass/Tile).

Reference computation:
    x1 = x @ W1                                    [N, F]
    w = softmax(where(seq > 0, 1, -9e15))          uniform over valid slots
    edge = relu(sum_l w[e,l] * x1[seq[e,l]])       [E, F]
    e1 = edge @ W2                                 [E, F]
    uw = softmax(where(useq > 0, 1, -9e15))
    node = sum_l uw[n,l] * e1[useq[n,l]]           [N, F]

Strategy (8-way SPMD), v6:
  - Slot-sum commutes with @W1: edge = relu(((1/c)*sum_l x[seq]) @ W1).
  - Gathers use the batched SWDGE dma_gather ucode op (mlp library): one
    instruction fetches 1024 rows (HW caps the idx-read free dim at 64
    int16-words/partition = 1024 idxs; bigger gathers raise
    illegal_instruction), vs the v4 per-slot indirect DMAs that paid
    ~1.1us SWDGE fixed cost per 128 rows.  Gathers round-robin the 4
    SWDGE queues so descriptor rings don't serialize back-to-back
    transfers.
  - dma_gather idxs are int16, so the x table can't be indexed directly
    (50001 rows > 32767).  Host builds per-core, per-tile-group compact
    tables: tiles are grouped (9/8/8), each group's referenced x rows
    (~26k unique < int16 range) are deduped into one table with a zero
    row for padding slots; indices are remapped to table-local.
  - Slot reduction is a single DVE tensor_reduce (fp32 accum) over a
    strided [128, 256, 32] view; the 1/count scale rides the ReLU
    activation (relu(r*z) = r*relu(z), r > 0).
  - Tables in bf16: 512B rows, half the HBM traffic of fp32.
  - e1 shard -> AllGather (bf16) -> stage-2 gather+reduce -> node out.
"""

import sys

sys.path.insert(0, "/opt/trn_rl_repo")

import numpy as np

N = 50000
E = 25000
F = 256
L = 32
P = 128
NC_COUNT = 8
NSH = N // NC_COUNT        # 6250 nodes per core
ESH = E // NC_COUNT        # 3125 edges per core
NSH_PAD = 6272             # 49 tiles
ESH_PAD = 3200             # 25 tiles
N_TILES_NODE = NSH_PAD // P
N_TILES_EDGE = ESH_PAD // P
GROUPS = [(0, 9), (9, 17), (17, 25)]   # stage-1 tile groups (compact tables)
NG = len(GROUPS)
TGROWS = 28672             # static rows per compact table (measured max ~26.2k)
XSENT = N                  # virtual id of the x zero row (padding slots)
EROWS = ESH + 1            # per-shard rows in the e1 table (incl zero row)
EZERO = ESH                # zero row lives in shard 0's slot ESH
NIDX = P * L               # 4096 indices per tile
GIDX = 1024                # idxs per dma_gather (HW cap)
NSPLIT = NIDX // GIDX      # 4 gathers per tile
IDXW = NIDX // 16          # 256 int16 idx words per partition per tile
GIDXW = GIDX // 16         # 64 idx words per gather
AG_SPLIT_T = 13            # e1 AllGather chunk A covers tiles [0, 13)
AG_ROWS = AG_SPLIT_T * P   # 1664 rows


def build_program():
    from concourse import bacc, bass, mybir, tile  # noqa: F401

    fp32 = mybir.dt.float32
    bf16 = mybir.dt.bfloat16
    i16 = mybir.dt.int16
    AL = mybir.AluOpType
    AF = mybir.ActivationFunctionType
    AX = mybir.AxisListType

    nc = bacc.Bacc("TRN2", target_bir_lowering=False, debug=False,
                   num_devices=NC_COUNT, num_swdge_queues=4)

    xg = [nc.dram_tensor(f"xg{g}", [TGROWS, F], bf16,
                         kind="ExternalInput").ap() for g in range(NG)]
    w1 = nc.dram_tensor("w1", [F, F], bf16, kind="ExternalInput").ap()
    w2 = nc.dram_tensor("w2", [F, F], bf16, kind="ExternalInput").ap()
    identd = nc.dram_tensor("identd", [P, P], bf16, kind="ExternalInput").ap()
    sidx1 = nc.dram_tensor("sidx1", [P, N_TILES_EDGE, IDXW], i16,
                           kind="ExternalInput").ap()
    sidx2 = nc.dram_tensor("sidx2", [P, N_TILES_NODE, IDXW], i16,
                           kind="ExternalInput").ap()
    srec1 = nc.dram_tensor("srec1", [P, N_TILES_EDGE], fp32,
                           kind="ExternalInput").ap()
    srec2 = nc.dram_tensor("srec2", [P, N_TILES_NODE], fp32,
                           kind="ExternalInput").ap()
    out = nc.dram_tensor("out", [NSH_PAD, F], fp32, kind="ExternalOutput").ap()

    qctr = [0]

    with tile.TileContext(nc) as tc:
        with (
            tc.tile_pool(name="cst", bufs=1) as cst,
            tc.tile_pool(name="gb", bufs=5) as gbp,
            tc.tile_pool(name="rd", bufs=3) as rdp,
            tc.tile_pool(name="sb", bufs=4) as sbp,
            tc.tile_pool(name="ps", bufs=4, space="PSUM") as psp,
            tc.tile_pool(name="pst", bufs=4, space="PSUM") as pstp,
            tc.tile_pool(name="dram", bufs=1, space="DRAM") as dram,
        ):
            # ---------- constants ----------
            ident = cst.tile([P, P], bf16)
            nc.sync.dma_start(out=ident[:], in_=identd[:, :])
            w1sb = [cst.tile([P, F], bf16, name=f"w1k{k}") for k in range(2)]
            w2sb = [cst.tile([P, F], bf16, name=f"w2k{k}") for k in range(2)]
            for k in range(2):
                nc.sync.dma_start(out=w1sb[k][:], in_=w1[k * P:(k + 1) * P, :])
                nc.sync.dma_start(out=w2sb[k][:], in_=w2[k * P:(k + 1) * P, :])
            zrow = cst.tile([1, F], bf16)
            nc.vector.memset(zrow[:], 0.0)

            i1 = cst.tile([P, N_TILES_EDGE, IDXW], i16, name="i1")
            i2 = cst.tile([P, N_TILES_NODE, IDXW], i16, name="i2")
            r1 = cst.tile([P, N_TILES_EDGE], fp32, name="r1")
            r2 = cst.tile([P, N_TILES_NODE], fp32, name="r2")
            nc.sync.dma_start(out=i1[:], in_=sidx1[:, :, :])
            nc.sync.dma_start(out=i2[:], in_=sidx2[:, :, :])
            nc.scalar.dma_start(out=r1[:], in_=srec1[:, :])
            nc.scalar.dma_start(out=r2[:], in_=srec2[:, :])

            # ---------- DRAM scratch ----------
            e1loc = dram.tile([EROWS, F], bf16)
            e1tab = dram.tile([NC_COUNT * EROWS, F], bf16,
                              addr_space="Shared")

            def gather_tile(g, table, idx_sb, t):
                """4x 1024-idx dma_gather into g[:, :L, :] (slot c of tile
                row p lands at g[p, c, :]); queues round-robin."""
                for k in range(NSPLIT):
                    nc.gpsimd.dma_gather(
                        out_ap=g[:, 8 * k:8 * (k + 1), :], in_ap=table,
                        idxs_ap=idx_sb[:, t, GIDXW * k:GIDXW * (k + 1)],
                        num_idxs=GIDX, num_idxs_reg=GIDX, elem_size=F,
                        queue_num=qctr[0] % 4)
                    qctr[0] += 1

            # ---------- stage 1: edges ----------
            with nc.named_scope("stage1"):
                for gi, (lo, hi) in enumerate(GROUPS):
                    for t in range(lo, hi):
                        g = gbp.tile([P, L, F], bf16, tag="g")
                        gather_tile(g, xg[gi][:], i1, t)
                        with nc.allow_low_precision(
                                reason="bf16 tree reduce; tolerance 2e-2"):
                            h = L
                            while h > 1:
                                h //= 2
                                nc.vector.tensor_tensor(
                                    out=g[:, 0:h, :], in0=g[:, 0:h, :],
                                    in1=g[:, h:2 * h, :], op=AL.add)
                        agg = g
                        # edge = relu(r1 * (agg @ W1)); e1 = edge @ W2
                        ps1 = psp.tile([P, F], fp32, tag="mm")
                        for kc in range(2):
                            pst = pstp.tile([P, P], bf16, tag="tr")
                            nc.tensor.transpose(
                                out=pst[:],
                                in_=agg[:, 0:1, kc * P:(kc + 1) * P].squeeze(1),
                                identity=ident[:])
                            aggT = sbp.tile([P, P], bf16, tag="aggT")
                            nc.scalar.activation(out=aggT[:], in_=pst[:],
                                                 func=AF.Copy)
                            nc.tensor.matmul(ps1[:], aggT[:], w1sb[kc][:],
                                             start=(kc == 0), stop=(kc == 1))
                        edge = sbp.tile([P, F], bf16, tag="edge")
                        nc.scalar.activation(out=edge[:], in_=ps1[:],
                                             func=AF.Relu,
                                             scale=r1[:, t:t + 1])
                        ps2 = psp.tile([P, F], fp32, tag="mm")
                        for kc in range(2):
                            pst = pstp.tile([P, P], bf16, tag="tr")
                            nc.tensor.transpose(
                                out=pst[:], in_=edge[:, kc * P:(kc + 1) * P],
                                identity=ident[:])
                            edgeT = sbp.tile([P, P], bf16, tag="edgeT")
                            nc.scalar.activation(out=edgeT[:], in_=pst[:],
                                                 func=AF.Copy)
                            nc.tensor.matmul(ps2[:], edgeT[:], w2sb[kc][:],
                                             start=(kc == 0), stop=(kc == 1))
                        pr = min(P, ESH - t * P)
                        if pr > 0:
                            e1sb = sbp.tile([P, F], bf16, tag="row")
                            nc.scalar.activation(out=e1sb[:], in_=ps2[:],
                                                 func=AF.Copy)
                            nc.sync.dma_start(
                                out=e1loc[t * P:t * P + pr, :],
                                in_=e1sb[:pr, :])
                nc.sync.dma_start(out=e1loc[ESH:ESH + 1, :], in_=zrow[:])
                nc.gpsimd.collective_compute(
                    "AllGather", AL.bypass,
                    replica_groups=[list(range(NC_COUNT))],
                    ins=[e1loc.opt()], outs=[e1tab.opt()],
                )

            # ---------- stage 2: nodes ----------
            with nc.named_scope("stage2"):
                for t in range(N_TILES_NODE):
                    g2 = gbp.tile([P, L, F], bf16, tag="g")
                    gather_tile(g2, e1tab[:], i2, t)
                    with nc.allow_low_precision(
                            reason="bf16 tree reduce; tolerance 2e-2"):
                        h = L
                        while h > 1:
                            h //= 2
                            nc.vector.tensor_tensor(
                                out=g2[:, 0:h, :], in0=g2[:, 0:h, :],
                                in1=g2[:, h:2 * h, :], op=AL.add)
                    node = sbp.tile([P, F], fp32, tag="node")
                    nc.scalar.activation(out=node[:], in_=g2[:, 0:1, :].squeeze(1),
                                         func=AF.Copy, scale=r2[:, t:t + 1])
                    nc.sync.dma_start(out=out[t * P:(t + 1) * P, :],
                                      in_=node[:])

    nc.compile()
    return nc


def _wrap_idxs(M, n_tiles):
    """[n_tiles*P, L] int -> [P, n_tiles, IDXW] int16 in dma_gather layout.

    Flat gather stream for tile t is j = c*128 + p -> M[t*P + p, c]
    (so out[p, c, :] = table[M[p, c]]); int16 idx words are wrapped into
    16 partitions (j at [j%16, j//16]) and replicated across the 8 q7
    cores (partition groups of 16).  1024-idx sub-gathers slice 64-word
    column blocks."""
    A = M.reshape(n_tiles, P, L).transpose(0, 2, 1)      # [t, c, p]
    W = A.reshape(n_tiles, IDXW, 16).transpose(0, 2, 1)  # [t, 16, IDXW]
    T = np.tile(W, (1, 8, 1))                            # [t, 128, IDXW]
    return np.ascontiguousarray(T.transpose(1, 0, 2)).astype(np.int16)


def _tile_recs(R, n_tiles):
    """[n_tiles*P] f32 -> [P, n_tiles] f32 (row t*P+p at [p, t])."""
    return np.ascontiguousarray(R.reshape(n_tiles, P).T)


def make_in_maps(x, seq, useq, W1, W2):
    import ml_dtypes

    bf16 = ml_dtypes.bfloat16
    x = np.asarray(x, dtype=np.float32)
    W1b = np.asarray(W1, dtype=np.float32).astype(bf16)
    W2b = np.asarray(W2, dtype=np.float32).astype(bf16)
    seq = np.asarray(seq)
    useq = np.asarray(useq)
    identb = np.eye(P, dtype=np.float32).astype(bf16)

    xe = np.zeros((N + 1, F), bf16)     # x rows + zero row at XSENT
    xe[:N] = x.astype(bf16)

    # uniform softmax weights: 1/count over valid (id>0) slots; all-padding
    # rows degenerate to 32 gathers of row 0 with weight 1/L
    m1 = seq > 0
    cnt1 = m1.sum(axis=1)
    rec1 = np.where(cnt1 > 0, 1.0 / np.maximum(cnt1, 1), 1.0 / L)
    rec1 = rec1.astype(np.float32)
    seq_m = np.where(m1, seq, np.where(cnt1[:, None] > 0, XSENT, 0))
    seq_m = seq_m.astype(np.int32)

    # e1 table layout: 8 shards x EROWS rows, zero row at slot ESH of shard 0
    m2 = useq > 0
    cnt2 = m2.sum(axis=1)
    rec2 = np.where(cnt2 > 0, 1.0 / np.maximum(cnt2, 1), 1.0 / L)
    rec2 = rec2.astype(np.float32)
    useq_m = np.where(m2, (useq // ESH) * EROWS + useq % ESH,
                      np.where(cnt2[:, None] > 0, EZERO, 0)).astype(np.int32)

    in_maps = []
    for c in range(NC_COUNT):
        s_c = np.full((ESH_PAD, L), XSENT, np.int32)
        s_c[:ESH] = seq_m[c * ESH:(c + 1) * ESH]
        r1 = np.zeros(ESH_PAD, np.float32)
        r1[:ESH] = rec1[c * ESH:(c + 1) * ESH]
        u_c = np.full((NSH_PAD, L), EZERO, np.int32)
        u_c[:NSH] = useq_m[c * NSH:(c + 1) * NSH]
        r2 = np.zeros(NSH_PAD, np.float32)
        r2[:NSH] = rec2[c * NSH:(c + 1) * NSH]

        # per-group compact tables + remapped local indices
        xtabs = np.zeros((NG, TGROWS, F), bf16)
        s_loc = np.empty_like(s_c)
        for gi, (lo, hi) in enumerate(GROUPS):
            blk = s_c[lo * P:hi * P]
            uniq = np.unique(np.append(blk.ravel(), XSENT))
            assert len(uniq) <= TGROWS, len(uniq)
            xtabs[gi, :len(uniq)] = xe[uniq]
            s_loc[lo * P:hi * P] = np.searchsorted(uniq, blk)
        # slot order is free (sum commutes): sort each row's indices so each
        # gather channel hits a narrow table band (HBM row locality)
        s_loc = np.sort(s_loc, axis=1)
        u_s = np.sort(u_c, axis=1)

        in_maps.append({
            **{f"xg{g}": np.ascontiguousarray(xtabs[g]) for g in range(NG)},
            "w1": W1b,
            "w2": W2b,
            "identd": identb,
            "sidx1": _wrap_idxs(s_loc, N_TILES_EDGE),
            "sidx2": _wrap_idxs(u_s, N_TILES_NODE),
            "srec1": _tile_recs(r1, N_TILES_EDGE),
            "srec2": _tile_recs(r2, N_TILES_NODE),
        })
    return in_maps


def kernel(x, seq, useq, W1, W2):
    from concourse.bass_utils import run_bass_kernel_spmd

    in_maps = make_in_maps(x, seq, useq, W1, W2)
    nc = build_program()
    res = run_bass_kernel_spmd(nc, in_maps, core_ids=list(range(NC_COUNT)),
                               trace=False)
    parts = [res.results[c]["out"][:NSH] for c in range(NC_COUNT)]
    return np.concatenate(parts, axis=0)
